# revision 42
# baseline (speedup 1.0000x reference)
"""HadLinear Trainium2 kernel: out = blockwise_FWHT(x)/sqrt(1024) @ w.T.

Strategy (8 NeuronCores, tensor-parallel over output features):
  - The blockwise Hadamard is linear: out = x @ V with V = B @ w.T and
    B = blockdiag(H_1024, x4) / 32 symmetric.  V is computed on-device
    using the Kronecker split H_1024 = H_8 (x) H_128:
      stage A (PE):  T1[kc] = (H_128/32) @ w_chunk[kc]   -- 32 matmuls,
                     one shared stationary, vs 256 matmuls for a direct
                     blockwise H_1024 product.
      stage B (DVE): 3 butterfly stages (+-) across the 8 chunks of each
                     1024-block combine T1 into V (the H_8 factor).
    Blocks are combined in order so the main matmul can begin as soon as
    block 0 of V is ready (block-ordered PSUM accumulation).
  - w is column-sharded: core c owns output features [c*512, (c+1)*512).
    Every core streams the full x (host-transposed to feature-major
    tiles of 512 tokens) and computes out[:, c*512:(c+1)*512].
  - DMA plan: weight blocks stage through the t2 scratch tile on the SP
    queue; x streams as per-block 1MB sub-DMAs on the (otherwise idle)
    GpSimd queue; outputs are batched per token-group.  This keeps any
    single queue from serializing behind a long transfer.
  - Matmuls run in bf16 with fp32 PSUM accumulation.
"""

import numpy as np
import ml_dtypes

import concourse.bacc as bacc
import concourse.tile as tile
import concourse.mybir as mybir
from concourse.bass_utils import run_bass_kernel_spmd

N_CORES = 8
B, S, D = 4, 2048, 4096          # input (B, S, D)
TOK = B * S                      # 8192 tokens
BLOCK = 1024                     # Hadamard block
OUT_PER_CORE = D // N_CORES      # 512 output features per core
K_CHUNKS = D // 128              # 32 contraction chunks
QR = BLOCK // 128                # 8 chunks per Hadamard block
N_BLOCKS = D // BLOCK            # 4 Hadamard blocks
G_TOK = 512                      # tokens per x tile
N_GROUPS = TOK // G_TOK          # 16 token groups
G_M = G_TOK // 128               # 4 output m-chunks per group
M_CHUNKS = TOK // 128            # 64 output chunks

BF16 = ml_dtypes.bfloat16

_PROGRAM = None


def _h128_table():
    """H[p, q] = H_128[p, q] / 32, bf16 (exact: entries are +-2^-5)."""
    idx = np.arange(128)
    anded = idx[:, None] & idx[None, :]
    par = np.zeros_like(anded)
    v = anded
    while v.any():
        par ^= v & 1
        v >>= 1
    return ((1 - 2 * par).astype(np.float32) / 32.0).astype(BF16)


def _q_idx(t, blk, q):
    """Index the chunk axis of a [128, 4, 2, 2, 2, 512] V-shaped tile."""
    return t[:, blk, (q >> 2) & 1, (q >> 1) & 1, q & 1, :]


def _build_program():
    nc = bacc.Bacc("TRN2", target_bir_lowering=False, debug=False,
                   num_devices=N_CORES)
    # xg[g, blk, p, q, t] = x[g*512 + t, blk*1024 + q*128 + p]
    x_d = nc.dram_tensor("xg", [N_GROUPS, N_BLOCKS, 128, QR, G_TOK],
                         mybir.dt.bfloat16, kind="ExternalInput")
    # wt[blk, p, q2, q1, q0, o] = w[c*512 + o, blk*1024 + (q2*4+q1*2+q0)*128 + p]
    w_d = nc.dram_tensor("wt", [N_BLOCKS, 128, 2, 2, 2, OUT_PER_CORE],
                         mybir.dt.bfloat16, kind="ExternalInput")
    h_d = nc.dram_tensor("h", [128, 128], mybir.dt.bfloat16,
                         kind="ExternalInput")
    # out[g, t, ml, o] = out_full[g*512 + ml*128 + t, c*512 + o]
    o_d = nc.dram_tensor("out", [N_GROUPS, 128, G_M, OUT_PER_CORE],
                         mybir.dt.bfloat16, kind="ExternalOutput")

    VSHAPE = [128, N_BLOCKS, 2, 2, 2, OUT_PER_CORE]

    with tile.TileContext(nc) as tc:
        with (
            tc.tile_pool(name="consts", bufs=1) as consts,
            tc.tile_pool(name="t1p", bufs=1) as t1p,
            tc.tile_pool(name="t2p", bufs=1) as t2p,
            tc.tile_pool(name="vp", bufs=1) as vp,
            tc.tile_pool(name="xin", bufs=2) as xin,
            tc.tile_pool(name="ost", bufs=2) as ost,
            tc.tile_pool(name="ps1", bufs=4, space="PSUM") as ps1,
            tc.tile_pool(name="ps2", bufs=1, space="PSUM") as ps2,
        ):
            h = consts.tile([128, 128], mybir.dt.bfloat16)
            nc.sync.dma_start(h[:], h_d[:])

            t1 = t1p.tile(VSHAPE, mybir.dt.bfloat16)
            t2 = t2p.tile(VSHAPE, mybir.dt.bfloat16)
            v = vp.tile(VSHAPE, mybir.dt.bfloat16)

            # Stage A + B per block: DMA w block into t2's block slice,
            # T1 = (H128/32) @ w_chunk on PE (evac to t1 via ACT), then the
            # three H8 butterfly stages on DVE: t1 -> t2 -> t1 -> v.
            for blk in range(N_BLOCKS):
                # split DMAs: the first T1 matmul starts earlier (block 0,
                # the stage-2 critical path, in quarters)
                if blk == 0:
                    # fine split across the SP, ACT and Pool queues: the
                    # last block-0 chunk's arrival gates the evac pipeline
                    # and thus V-block-0 (the stage-2 open time).  Pool's
                    # first x-tile isn't consumed until ~12us, so chunks
                    # 6-7 jump ahead of it on that queue.
                    nc.sync.dma_start(t2[:, 0, 0, 0, 0, :], w_d[0, :, 0, 0, 0])
                    nc.sync.dma_start(t2[:, 0, 0, 0, 1, :], w_d[0, :, 0, 0, 1])
                    nc.gpsimd.dma_start(t2[:, 0, 0, 1, 0, :], w_d[0, :, 0, 1, 0])
                    nc.gpsimd.dma_start(t2[:, 0, 0, 1, 1, :], w_d[0, :, 0, 1, 1])
                    nc.scalar.dma_start(t2[:, 0, 1, 0, 0, :], w_d[0, :, 1, 0, 0])
                    nc.scalar.dma_start(t2[:, 0, 1, 0, 1, :], w_d[0, :, 1, 0, 1])
                    nc.gpsimd.dma_start(t2[:, 0, 1, 1, :, :], w_d[0, :, 1, 1])
                else:
                    nc.sync.dma_start(t2[:, blk, 0, :, :, :], w_d[blk, :, 0])
                    nc.sync.dma_start(t2[:, blk, 1, :, :, :], w_d[blk, :, 1])
                for q in range(QR):
                    acc = ps1.tile([128, OUT_PER_CORE], mybir.dt.float32)
                    nc.tensor.matmul(acc[:], h[:], _q_idx(t2, blk, q),
                                     start=True, stop=True)
                    # ACT's ~610ns/chunk evac cadence bounds V-block
                    # latency; for block 0 (the stage-2 critical path)
                    # alternate DVE in for odd chunks.
                    if blk == 0 and (q & 1):
                        nc.vector.tensor_copy(out=_q_idx(t1, blk, q),
                                              in_=acc[:])
                    else:
                        nc.scalar.copy(_q_idx(t1, blk, q), acc[:])
                # butterfly on chunk bit 0: t1 -> t2 (overwrites w staging)
                a = t1[:, blk, :, :, 0, :]
                b = t1[:, blk, :, :, 1, :]
                nc.vector.tensor_tensor(t2[:, blk, :, :, 0, :], a, b,
                                        mybir.AluOpType.add)
                nc.vector.tensor_tensor(t2[:, blk, :, :, 1, :], a, b,
                                        mybir.AluOpType.subtract)
                # bit 1: t2 -> t1
                a = t2[:, blk, :, 0, :, :]
                b = t2[:, blk, :, 1, :, :]
                nc.vector.tensor_tensor(t1[:, blk, :, 0, :, :], a, b,
                                        mybir.AluOpType.add)
                nc.vector.tensor_tensor(t1[:, blk, :, 1, :, :], a, b,
                                        mybir.AluOpType.subtract)
                # bit 2: t1 -> v
                a = t1[:, blk, 0, :, :, :]
                b = t1[:, blk, 1, :, :, :]
                nc.vector.tensor_tensor(v[:, blk, 0, :, :, :], a, b,
                                        mybir.AluOpType.add)
                nc.vector.tensor_tensor(v[:, blk, 1, :, :, :], a, b,
                                        mybir.AluOpType.subtract)

            # Stage 2: out[g] = X[g] @ V, block-ordered accumulation so the
            # first groups can start before all of V is combined.
            for g in range(N_GROUPS):
                last_g = g == N_GROUPS - 1
                xg = xin.tile([128, K_CHUNKS, G_TOK], mybir.dt.bfloat16)
                for blk in range(N_BLOCKS):
                    nc.gpsimd.dma_start(
                        xg[:, blk * QR:(blk + 1) * QR, :], x_d[g, blk])
                accs = [ps2.tile([128, OUT_PER_CORE], mybir.dt.float32,
                                 name=f"acc{ml}", tag=f"acc{ml}")
                        for ml in range(G_M)]
                for blk in range(N_BLOCKS):
                    for ml in range(G_M):
                        for q in range(QR):
                            kc = blk * QR + q
                            nc.tensor.matmul(
                                accs[ml][:],
                                xg[:, kc, ml * 128:(ml + 1) * 128],
                                _q_idx(v, blk, q),
                                start=(blk == 0 and q == 0),
                                stop=(blk == N_BLOCKS - 1 and q == QR - 1),
                            )
                ot = ost.tile([128, G_M, OUT_PER_CORE], mybir.dt.bfloat16)
                if last_g:
                    for ml in range(G_M):
                        nc.scalar.copy(ot[:, ml, :], accs[ml][:])
                        nc.scalar.dma_start(o_d[g, :, ml, :], ot[:, ml, :])
                else:
                    for ml in range(G_M):
                        nc.scalar.copy(ot[:, ml, :], accs[ml][:])
                    nc.sync.dma_start(o_d[g], ot[:])

    nc.compile()
    return nc


def _get_program():
    global _PROGRAM
    if _PROGRAM is None:
        _PROGRAM = _build_program()
    return _PROGRAM


def _prep_inputs(input, weight):
    x = np.asarray(input, dtype=np.float32).reshape(TOK, D)
    w = np.asarray(weight, dtype=np.float32)
    # xg[g, blk, p, q, t] = x[g*512 + t, blk*1024 + q*128 + p]
    xg = np.ascontiguousarray(
        x.reshape(N_GROUPS, G_TOK, N_BLOCKS, QR, 128).transpose(0, 2, 4, 3, 1)
    ).astype(BF16)
    h = _h128_table()
    in_maps = []
    for c in range(N_CORES):
        wsl = w[c * OUT_PER_CORE:(c + 1) * OUT_PER_CORE, :]  # [512, 4096]
        # wt[blk, p, q, o] = wsl.T[blk*1024 + q*128 + p, o]
        wt = np.ascontiguousarray(
            wsl.T.reshape(N_BLOCKS, QR, 128, OUT_PER_CORE).transpose(0, 2, 1, 3)
        ).reshape(N_BLOCKS, 128, 2, 2, 2, OUT_PER_CORE).astype(BF16)
        in_maps.append({"xg": xg, "wt": wt, "h": h})
    return in_maps


def kernel(input, weight):
    import time as _time

    nc = _get_program()
    in_maps = _prep_inputs(input, weight)
    # The axon-side XLA compile of the bass_exec custom call is
    # intermittently flaky (CallFunctionObjArgs INTERNAL error) on first
    # compile in a fresh process; a clean retry re-lowers and succeeds.
    last_exc = None
    for attempt in range(3):
        try:
            res = run_bass_kernel_spmd(nc, in_maps, list(range(N_CORES)))
            break
        except Exception as exc:  # noqa: BLE001 - retry transient compile/exec
            # Also rides out a stale device wedge (NRT_EXEC_UNIT_UNRECOVERABLE),
            # which clears on a ~1-2 minute timescale.
            last_exc = exc
            _time.sleep(30.0 * (attempt + 1))
    else:
        raise last_exc
    # out[g, t, ml, o] -> [tok, o]
    parts = [res.results[c]["out"].astype(np.float32).transpose(0, 2, 1, 3)
             .reshape(TOK, OUT_PER_CORE) for c in range(N_CORES)]
    out = np.concatenate(parts, axis=1).reshape(B, S, D)
    return np.ascontiguousarray(out, dtype=np.float32)


# revision 43
# speedup vs baseline: 1.0003x; 1.0003x over previous
"""HadLinear Trainium2 kernel: out = blockwise_FWHT(x)/sqrt(1024) @ w.T.

Strategy (8 NeuronCores, tensor-parallel over output features):
  - The blockwise Hadamard is linear: out = x @ V with V = B @ w.T and
    B = blockdiag(H_1024, x4) / 32 symmetric.  V is computed on-device
    using the Kronecker split H_1024 = H_8 (x) H_128:
      stage A (PE):  T1[kc] = (H_128/32) @ w_chunk[kc]   -- 32 matmuls,
                     one shared stationary, vs 256 matmuls for a direct
                     blockwise H_1024 product.
      stage B (DVE): 3 butterfly stages (+-) across the 8 chunks of each
                     1024-block combine T1 into V (the H_8 factor).
    Blocks are combined in order so the main matmul can begin as soon as
    block 0 of V is ready (block-ordered PSUM accumulation).
  - w is column-sharded: core c owns output features [c*512, (c+1)*512).
    Every core streams the full x (host-transposed to feature-major
    tiles of 512 tokens) and computes out[:, c*512:(c+1)*512].
  - DMA plan: weight blocks stage through the t2 scratch tile on the SP
    queue; x streams as per-block 1MB sub-DMAs on the (otherwise idle)
    GpSimd queue; outputs are batched per token-group.  This keeps any
    single queue from serializing behind a long transfer.
  - Matmuls run in bf16 with fp32 PSUM accumulation.
"""

import numpy as np
import ml_dtypes

import concourse.bacc as bacc
import concourse.tile as tile
import concourse.mybir as mybir
from concourse.bass_utils import run_bass_kernel_spmd

N_CORES = 8
B, S, D = 4, 2048, 4096          # input (B, S, D)
TOK = B * S                      # 8192 tokens
BLOCK = 1024                     # Hadamard block
OUT_PER_CORE = D // N_CORES      # 512 output features per core
K_CHUNKS = D // 128              # 32 contraction chunks
QR = BLOCK // 128                # 8 chunks per Hadamard block
N_BLOCKS = D // BLOCK            # 4 Hadamard blocks
G_TOK = 512                      # tokens per x tile
N_GROUPS = TOK // G_TOK          # 16 token groups
G_M = G_TOK // 128               # 4 output m-chunks per group
M_CHUNKS = TOK // 128            # 64 output chunks

BF16 = ml_dtypes.bfloat16

_PROGRAM = None


def _h128_table():
    """H[p, q] = H_128[p, q] / 32, bf16 (exact: entries are +-2^-5)."""
    idx = np.arange(128)
    anded = idx[:, None] & idx[None, :]
    par = np.zeros_like(anded)
    v = anded
    while v.any():
        par ^= v & 1
        v >>= 1
    return ((1 - 2 * par).astype(np.float32) / 32.0).astype(BF16)


def _q_idx(t, blk, q):
    """Index the chunk axis of a [128, 4, 2, 2, 2, 512] V-shaped tile."""
    return t[:, blk, (q >> 2) & 1, (q >> 1) & 1, q & 1, :]


def _build_program():
    nc = bacc.Bacc("TRN2", target_bir_lowering=False, debug=False,
                   num_devices=N_CORES)
    # xg[g, blk, p, q, t] = x[g*512 + t, blk*1024 + q*128 + p]
    x_d = nc.dram_tensor("xg", [N_GROUPS, N_BLOCKS, 128, QR, G_TOK],
                         mybir.dt.bfloat16, kind="ExternalInput")
    # wt[blk, p, q2, q1, q0, o] = w[c*512 + o, blk*1024 + (q2*4+q1*2+q0)*128 + p]
    w_d = nc.dram_tensor("wt", [N_BLOCKS, 128, 2, 2, 2, OUT_PER_CORE],
                         mybir.dt.bfloat16, kind="ExternalInput")
    h_d = nc.dram_tensor("h", [128, 128], mybir.dt.bfloat16,
                         kind="ExternalInput")
    # out[g, t, ml, o] = out_full[g*512 + ml*128 + t, c*512 + o]
    o_d = nc.dram_tensor("out", [N_GROUPS, 128, G_M, OUT_PER_CORE],
                         mybir.dt.bfloat16, kind="ExternalOutput")

    VSHAPE = [128, N_BLOCKS, 2, 2, 2, OUT_PER_CORE]

    with tile.TileContext(nc) as tc:
        with (
            tc.tile_pool(name="consts", bufs=1) as consts,
            tc.tile_pool(name="t1p", bufs=1) as t1p,
            tc.tile_pool(name="t2p", bufs=1) as t2p,
            tc.tile_pool(name="vp", bufs=1) as vp,
            tc.tile_pool(name="xin", bufs=2) as xin,
            tc.tile_pool(name="ost", bufs=2) as ost,
            tc.tile_pool(name="ps1", bufs=4, space="PSUM") as ps1,
            tc.tile_pool(name="ps2", bufs=1, space="PSUM") as ps2,
        ):
            h = consts.tile([128, 128], mybir.dt.bfloat16)
            nc.sync.dma_start(h[:], h_d[:])

            t1 = t1p.tile(VSHAPE, mybir.dt.bfloat16)
            t2 = t2p.tile(VSHAPE, mybir.dt.bfloat16)
            v = vp.tile(VSHAPE, mybir.dt.bfloat16)

            # Stage A + B per block: DMA w block into t2's block slice,
            # T1 = (H128/32) @ w_chunk on PE (evac to t1 via ACT), then the
            # three H8 butterfly stages on DVE: t1 -> t2 -> t1 -> v.
            for blk in range(N_BLOCKS):
                # split DMAs: the first T1 matmul starts earlier (block 0,
                # the stage-2 critical path, in quarters)
                if blk == 0:
                    # fine split across the SP, ACT and Pool queues: the
                    # last block-0 chunk's arrival gates the evac pipeline
                    # and thus V-block-0 (the stage-2 open time).  Pool's
                    # first x-tile isn't consumed until ~12us, so chunks
                    # 6-7 jump ahead of it on that queue.
                    nc.sync.dma_start(t2[:, 0, 0, 0, 0, :], w_d[0, :, 0, 0, 0])
                    nc.sync.dma_start(t2[:, 0, 0, 0, 1, :], w_d[0, :, 0, 0, 1])
                    nc.gpsimd.dma_start(t2[:, 0, 0, 1, 0, :], w_d[0, :, 0, 1, 0])
                    nc.gpsimd.dma_start(t2[:, 0, 0, 1, 1, :], w_d[0, :, 0, 1, 1])
                    nc.scalar.dma_start(t2[:, 0, 1, 0, 0, :], w_d[0, :, 1, 0, 0])
                    nc.scalar.dma_start(t2[:, 0, 1, 0, 1, :], w_d[0, :, 1, 0, 1])
                    nc.gpsimd.dma_start(t2[:, 0, 1, 1, :, :], w_d[0, :, 1, 1])
                else:
                    nc.sync.dma_start(t2[:, blk, 0, :, :, :], w_d[blk, :, 0])
                    nc.sync.dma_start(t2[:, blk, 1, :, :, :], w_d[blk, :, 1])
                for q in range(QR):
                    acc = ps1.tile([128, OUT_PER_CORE], mybir.dt.float32)
                    nc.tensor.matmul(acc[:], h[:], _q_idx(t2, blk, q),
                                     start=True, stop=True)
                    # ACT's ~610ns/chunk evac cadence bounds V-block
                    # latency; for block 0 (the stage-2 critical path)
                    # alternate DVE in for odd chunks.
                    if blk == 0 and (q & 1):
                        nc.vector.tensor_copy(out=_q_idx(t1, blk, q),
                                              in_=acc[:])
                    else:
                        nc.scalar.copy(_q_idx(t1, blk, q), acc[:])
                # butterfly on chunk bit 0: t1 -> t2 (overwrites w staging)
                a = t1[:, blk, :, :, 0, :]
                b = t1[:, blk, :, :, 1, :]
                nc.vector.tensor_tensor(t2[:, blk, :, :, 0, :], a, b,
                                        mybir.AluOpType.add)
                nc.vector.tensor_tensor(t2[:, blk, :, :, 1, :], a, b,
                                        mybir.AluOpType.subtract)
                # bit 1: t2 -> t1
                a = t2[:, blk, :, 0, :, :]
                b = t2[:, blk, :, 1, :, :]
                nc.vector.tensor_tensor(t1[:, blk, :, 0, :, :], a, b,
                                        mybir.AluOpType.add)
                nc.vector.tensor_tensor(t1[:, blk, :, 1, :, :], a, b,
                                        mybir.AluOpType.subtract)
                # bit 2: t1 -> v (block 0's add in halves: the first two V
                # chunks unlock stage 2 one DVE op earlier)
                a = t1[:, blk, 0, :, :, :]
                b = t1[:, blk, 1, :, :, :]
                if blk == 0:
                    for q1 in (0, 1):
                        nc.vector.tensor_tensor(
                            v[:, 0, 0, q1, :, :], t1[:, 0, 0, q1, :, :],
                            t1[:, 0, 1, q1, :, :], mybir.AluOpType.add)
                else:
                    nc.vector.tensor_tensor(v[:, blk, 0, :, :, :], a, b,
                                            mybir.AluOpType.add)
                nc.vector.tensor_tensor(v[:, blk, 1, :, :, :], a, b,
                                        mybir.AluOpType.subtract)

            # Stage 2: out[g] = X[g] @ V, block-ordered accumulation so the
            # first groups can start before all of V is combined.
            for g in range(N_GROUPS):
                last_g = g == N_GROUPS - 1
                xg = xin.tile([128, K_CHUNKS, G_TOK], mybir.dt.bfloat16)
                for blk in range(N_BLOCKS):
                    nc.gpsimd.dma_start(
                        xg[:, blk * QR:(blk + 1) * QR, :], x_d[g, blk])
                accs = [ps2.tile([128, OUT_PER_CORE], mybir.dt.float32,
                                 name=f"acc{ml}", tag=f"acc{ml}")
                        for ml in range(G_M)]
                for blk in range(N_BLOCKS):
                    for ml in range(G_M):
                        for q in range(QR):
                            kc = blk * QR + q
                            nc.tensor.matmul(
                                accs[ml][:],
                                xg[:, kc, ml * 128:(ml + 1) * 128],
                                _q_idx(v, blk, q),
                                start=(blk == 0 and q == 0),
                                stop=(blk == N_BLOCKS - 1 and q == QR - 1),
                            )
                ot = ost.tile([128, G_M, OUT_PER_CORE], mybir.dt.bfloat16)
                if last_g:
                    for ml in range(G_M):
                        nc.scalar.copy(ot[:, ml, :], accs[ml][:])
                        nc.scalar.dma_start(o_d[g, :, ml, :], ot[:, ml, :])
                else:
                    for ml in range(G_M):
                        nc.scalar.copy(ot[:, ml, :], accs[ml][:])
                    nc.sync.dma_start(o_d[g], ot[:])

    nc.compile()
    return nc


def _get_program():
    global _PROGRAM
    if _PROGRAM is None:
        _PROGRAM = _build_program()
    return _PROGRAM


def _prep_inputs(input, weight):
    x = np.asarray(input, dtype=np.float32).reshape(TOK, D)
    w = np.asarray(weight, dtype=np.float32)
    # xg[g, blk, p, q, t] = x[g*512 + t, blk*1024 + q*128 + p]
    xg = np.ascontiguousarray(
        x.reshape(N_GROUPS, G_TOK, N_BLOCKS, QR, 128).transpose(0, 2, 4, 3, 1)
    ).astype(BF16)
    h = _h128_table()
    in_maps = []
    for c in range(N_CORES):
        wsl = w[c * OUT_PER_CORE:(c + 1) * OUT_PER_CORE, :]  # [512, 4096]
        # wt[blk, p, q, o] = wsl.T[blk*1024 + q*128 + p, o]
        wt = np.ascontiguousarray(
            wsl.T.reshape(N_BLOCKS, QR, 128, OUT_PER_CORE).transpose(0, 2, 1, 3)
        ).reshape(N_BLOCKS, 128, 2, 2, 2, OUT_PER_CORE).astype(BF16)
        in_maps.append({"xg": xg, "wt": wt, "h": h})
    return in_maps


def kernel(input, weight):
    import time as _time

    nc = _get_program()
    in_maps = _prep_inputs(input, weight)
    # The axon-side XLA compile of the bass_exec custom call is
    # intermittently flaky (CallFunctionObjArgs INTERNAL error) on first
    # compile in a fresh process; a clean retry re-lowers and succeeds.
    last_exc = None
    for attempt in range(3):
        try:
            res = run_bass_kernel_spmd(nc, in_maps, list(range(N_CORES)))
            break
        except Exception as exc:  # noqa: BLE001 - retry transient compile/exec
            # Also rides out a stale device wedge (NRT_EXEC_UNIT_UNRECOVERABLE),
            # which clears on a ~1-2 minute timescale.
            last_exc = exc
            _time.sleep(30.0 * (attempt + 1))
    else:
        raise last_exc
    # out[g, t, ml, o] -> [tok, o]
    parts = [res.results[c]["out"].astype(np.float32).transpose(0, 2, 1, 3)
             .reshape(TOK, OUT_PER_CORE) for c in range(N_CORES)]
    out = np.concatenate(parts, axis=1).reshape(B, S, D)
    return np.ascontiguousarray(out, dtype=np.float32)
